# revision 39
# baseline (speedup 1.0000x reference)
"""AxialAttention TRN2 Bass kernel (v2).

Shapes (hardcoded): x [B=4,T=16,C=256,H=64,W=64] fp32.
N = B*T*H = 4096 lines of [L=64, C=256]; heads=8, d=32.
Sharding: 64 (b,t) blocks -> 8 per core across 8 cores.

Per-core dataflow, per (b,t) block (xT = x[b,t] viewed [C=256, HW=4096]):
  qkT  = w_qk^T @ xT    (fp32r MMs, q pre-scaled, b_q folded at evac)
  v    = xT^T @ w_v     (row-major v; v_sw = partition-swapped copy via DMA)
  vbd  = block-diagonal packed v tiles [128, 64] per (line, pair) built by
         DVE strided copies; pair p = heads (p, p+4)
  per 8-line group, per pair b:
    sps[128,512] = seed MM (bias pair-lhsT [64,128] @ tiled-I) ; += kT^T qT
    probs = exp(sps)  (ACT, bf16)
    Z    = ez^T @ probs  (psum [8, 512])
    rz   = reciprocal_approx_fast(Z)  (DVE, f32)
    rbc  = ebc^T @ rz    (broadcast over 32-part slots, psum [128,512] x2)
    oT   = vbd^T @ probs (pair-packed MMs, M=64, K=128) -> ops [128,1024]
    oTsb = ops * rbc     (DVE, psum x psum -> f32r sbuf)
    outT = w_proj'^T @ oTsb + b'  (w_proj rows permuted to oT head order)
  out-projection of group g is emitted inside group g+1 (software pipeline)
  b_k dropped (softmax shift-invariant); b_v folded into b'.
"""

import numpy as np

B, T, C, H, W = 4, 16, 256, 64, 64
HEADS, D = 8, 32
NBT = B * T            # 64 (b,t) blocks
NCORES = 8
BT_PER_CORE = NBT // NCORES  # 8
HW = H * W             # 4096 positions per block
L = W                  # 64
GRP = 8                # lines per attention group
NGRP = H // GRP        # 8 groups per block
GQ = GRP * L           # 512 free columns per group

STW = 4384             # bf16 statics width (cols)


def _build_bass():
    import concourse.bacc as bacc
    import concourse.mybir as mybir
    from concourse.tile import TileContext

    f32 = mybir.dt.float32
    f32r = mybir.dt.float32r
    bf16 = mybir.dt.bfloat16
    AF = mybir.ActivationFunctionType

    nc = bacc.Bacc("TRN2", target_bir_lowering=False, debug=False,
                   num_devices=NCORES)

    x_d = nc.dram_tensor("x", [BT_PER_CORE, C, HW], bf16, kind="ExternalInput").ap()
    st_d = nc.dram_tensor("statics", [128, STW], bf16, kind="ExternalInput").ap()
    stf_d = nc.dram_tensor("staticf", [128, 4], f32, kind="ExternalInput").ap()
    out_d = nc.dram_tensor("out", [BT_PER_CORE, C, HW], f32, kind="ExternalOutput").ap()

    with TileContext(nc) as tc:
        with (
            tc.tile_pool(name="static", bufs=1) as stat,
            tc.tile_pool(name="xt", bufs=2) as pxt,
            tc.tile_pool(name="qk", bufs=8) as pqk,
            tc.tile_pool(name="vsb", bufs=2) as pv,
            tc.tile_pool(name="probs", bufs=2) as ppr,
            tc.tile_pool(name="eraw", bufs=2) as per,
            tc.tile_pool(name="zsb", bufs=2) as pz,
            tc.tile_pool(name="osb", bufs=4) as po,
            tc.tile_pool(name="outsb", bufs=4) as pout,
            tc.tile_pool(name="psS", bufs=2, space="PSUM") as psS,
            tc.tile_pool(name="psZ", bufs=1, space="PSUM") as psZ,
            tc.tile_pool(name="psAV", bufs=1, space="PSUM") as psAV,
            tc.tile_pool(name="psM", bufs=3, space="PSUM") as psM,
        ):
            # ---- static loads: bf16 constants + tiny f32 biases ----
            st = stat.tile([128, STW], bf16, tag="st", name="statics_sb")
            nc.sync.dma_start(out=st, in_=st_d)
            stf = stat.tile([128, 4], f32, tag="stf", name="staticf_sb")
            nc.sync.dma_start(out=stf, in_=stf_d)
            wqk = [st[:, 512 * i:512 * (i + 1)] for i in range(2)]
            wv = [st[:, 1024 + 256 * i:1024 + 256 * (i + 1)] for i in range(2)]
            wp = [[st[:, 1536 + 256 * i + 128 * j:1536 + 256 * i + 128 * (j + 1)]
                   for j in range(2)] for i in range(2)]
            expb = st[:, 2048:4096]
            ez = st[:, 4096:4128]
            ebc_bf = st[0:8, 4128:4384]
            bq = stf[:, 0:2]
            bp = stf[:, 2:4]
            # block-diag packed v tiles (zeros are structural, set once)
            vbd = [stat.tile([128, 2048], bf16, tag=f"vbd{i}", name=f"vbd{i}")
                   for i in range(2)]
            for t in vbd:
                nc.vector.memset(t, 0.0)

            def load_x(bt):
                xt = [pxt.tile([128, HW], bf16, tag="xt", name="xt") for _ in range(2)]
                for kc in range(2):
                    nc.sync.dma_start(out=xt[kc], in_=x_d[bt, 128 * kc:128 * (kc + 1), :])
                return xt

            def proj_gen(bt, xt, blk):
                """Emit block bt's projections in small chunks (yield points)
                so they can be dripped into the previous block's attention
                groups. Fills `blk` with v_sb/v_sw/qkT for block bt."""
                # ---- v projection (row-major): v_sb [128, 8192] ----
                v_sb = pv.tile([128, 8192], bf16, tag="vsb", name="vsb")
                blk["v_sb"] = v_sb
                for pt in range(16):  # two position-chunks (4 lines) per tile
                    ps = psM.tile([128, 512], f32, tag="mm", name="psmmv")
                    for sub in range(2):
                        pc = 2 * pt + sub
                        for kc in range(2):
                            nc.tensor.matmul(
                                ps[:, 256 * sub:256 * (sub + 1)],
                                xt[kc][:, 128 * pc:128 * (pc + 1)],
                                wv[kc], start=(kc == 0), stop=(kc == 1))
                    dst = v_sb[:, 512 * pt:512 * (pt + 1)]
                    if pt % 2 == 0:
                        nc.scalar.copy(dst, ps)
                    else:
                        nc.vector.tensor_copy(dst, ps)
                    yield

                # partition-swapped v copy (for odd/even line alignment)
                v_sw = pv.tile([128, 8192], bf16, tag="vsw", name="vsw")
                blk["v_sw"] = v_sw
                nc.sync.dma_start(out=v_sw[0:64, :], in_=v_sb[64:128, :])
                nc.sync.dma_start(out=v_sw[64:128, :], in_=v_sb[0:64, :])

                # ---- qk projection: qkT [512, 4096] -> bf16 sbuf ----
                qkT = [pqk.tile([128, HW], bf16, tag="qkT", name="qkT")
                       for _ in range(4)]
                blk["qkT"] = qkT
                for mc in range(4):
                    for nn in range(8):
                        ps = psM.tile([128, 512], f32, tag="mm", name="psmm")
                        for kc in range(2):
                            nc.tensor.matmul(
                                ps, wqk[kc][:, 128 * mc:128 * (mc + 1)],
                                xt[kc][:, 512 * nn:512 * (nn + 1)],
                                start=(kc == 0), stop=(kc == 1))
                        dst = qkT[mc][:, 512 * nn:512 * (nn + 1)]
                        if mc < 2:  # q: fold b_q (pre-scaled), ACT
                            nc.scalar.activation(dst, ps, AF.Identity,
                                                 bias=bq[:, mc:mc + 1], scale=1.0)
                        else:       # k: plain copy, DVE
                            nc.vector.tensor_copy(dst, ps)
                        if mc == 3 and nn == 3 and bt + 1 < BT_PER_CORE:
                            blk["xt_next"] = load_x(bt + 1)
                        yield

            pend = None  # deferred out-projection (oT tiles, group, block)
            tail = None  # previous group's state (probs/vbd), tail pipelined

            # block 0's projections run up front
            xt0 = load_x(0)
            blk = {}
            for _ in proj_gen(0, xt0, blk):
                pass
            gen_next = None

            for bt in range(BT_PER_CORE):
                v_sb, v_sw, qkT = blk["v_sb"], blk["v_sw"], blk["qkT"]
                xt_next = blk.get("xt_next")
                if bt + 1 < BT_PER_CORE:
                    blk_next = {}
                    gen_next = proj_gen(bt + 1, xt_next, blk_next)
                else:
                    gen_next = None

                # 4-D views for vbd construction
                # v_sb col = 256*pc + 32*head + d ; view [128, pc, head, d]
                vsb4 = v_sb.rearrange("p (a h c) -> p a h c", h=8, c=32)
                vsw4 = v_sw.rearrange("p (a h c) -> p a h c", h=8, c=32)

                for g in range(NGRP):
                    l0 = g * GRP
                    vb = vbd[g % 2]
                    # vbd col = 256*li + 64*pair + 32*lo + c
                    vb6 = vb.rearrange("p (j a b lo c) -> p j a b lo c",
                                       a=2, b=4, lo=2, c=32)
                    # (parity a, lo): upper(lo=0) rows 0:64 = head b of line;
                    # lower(lo=1) rows 64:128 = head b+4 of line
                    for a in range(2):
                        for lo in range(2):
                            src_t = vsb4 if (a == 0) == (lo == 0) else vsw4
                            srows = slice(0, 64) if lo == 0 else slice(64, 128)
                            src = src_t[srows, 4 * g:4 * g + 4,
                                        4 * lo:4 * lo + 4, :]
                            dst = vb6[srows, :, a, :, lo, :]
                            nc.vector.tensor_copy(dst, src)

                    # --- tail of the PREVIOUS group: Z matmuls + recip.
                    # Its probs/exp/mult are a full group old, so nothing
                    # here stalls the PE; recip lands on DVE ahead of this
                    # group's probs-multiplies.
                    if tail is not None:
                        tprobs = tail["probs"]
                        zps = psZ.tile([8, GQ], f32, tag="z", name="psz")
                        for b in range(4):
                            nc.tensor.matmul(
                                zps, ez[:, 8 * b:8 * (b + 1)],
                                tprobs[:, GQ * b:GQ * (b + 1)],
                                start=(b == 0), stop=(b == 3))
                        rz_f = pz.tile([8, GQ], f32, tag="zf", name="rzf")
                        nc.vector.reciprocal_approx_fast(out=rz_f, in_=zps)
                        z_sb = pz.tile([8, GQ], bf16, tag="z", name="zsb")
                        with nc.allow_low_precision(reason="bf16 1/Z"):
                            nc.scalar.copy(z_sb, rz_f)
                        tail["z_sb"] = z_sb

                    # --- this group's scores / exp / exp-bias multiply
                    probs = ppr.tile([128, 4 * GQ], bf16, tag="probs", name="probs")
                    for b in range(4):
                        sp = psS.tile([128, GQ], f32, tag="s", name="pss")
                        for li in range(GRP):
                            l = l0 + li
                            for hh in range(2):  # head b + 4*hh
                                kt = qkT[2 + hh][32 * b:32 * (b + 1),
                                                 64 * l:64 * (l + 1)]
                                qt = qkT[hh][32 * b:32 * (b + 1),
                                             64 * l:64 * (l + 1)]
                                nc.tensor.matmul(
                                    sp[64 * hh:64 * (hh + 1),
                                       64 * li:64 * (li + 1)],
                                    kt, qt, start=True, stop=True,
                                    tile_position=(32 * b, 64 * hh))
                        eraw = per.tile([128, GQ], bf16, tag="eraw", name="eraw")
                        nc.scalar.activation(eraw, sp, AF.Exp, scale=1.0)
                        with nc.allow_low_precision(reason="probs are bf16"):
                            nc.gpsimd.tensor_mul(
                                probs[:, GQ * b:GQ * (b + 1)], eraw,
                                expb[:, GQ * b:GQ * (b + 1)])

                    # --- rest of the previous group's tail: attnv, rbc,
                    # normalize; then the out-projection two groups back
                    if tail is not None:
                        tprobs, tvb, z_sb = tail["probs"], tail["vb"], tail["z_sb"]
                        ops = psAV.tile([128, 2 * GQ], f32, tag="ops",
                                        name="psops")
                        for c in range(2):
                            for li in range(GRP):
                                for pb in (2 * c, 2 * c + 1):
                                    nc.tensor.matmul(
                                        ops[64 * (pb % 2):64 * (pb % 2 + 1),
                                            GQ * c + 64 * li:
                                            GQ * c + 64 * (li + 1)],
                                        tvb[:, 256 * li + 64 * pb:
                                            256 * li + 64 * (pb + 1)],
                                        tprobs[:, GQ * pb + 64 * li:
                                               GQ * pb + 64 * (li + 1)],
                                        start=True, stop=True)
                        rbc = [psM.tile([128, GQ], f32, tag="mm", name="psrbc")
                               for _ in range(2)]
                        for c in range(2):
                            nc.tensor.matmul(rbc[c],
                                             ebc_bf[:, 128 * c:128 * (c + 1)],
                                             z_sb, start=True, stop=True)
                        rbc_sb = [po.tile([128, GQ], f32, tag="rbcsb",
                                          name="rbcsb") for _ in range(2)]
                        nc.scalar.copy(rbc_sb[0], rbc[0])
                        nc.vector.tensor_copy(rbc_sb[1], rbc[1])
                        oT = [po.tile([128, GQ], bf16, tag="oT", name="oT")
                              for _ in range(2)]
                        with nc.allow_low_precision(reason="bf16 o"):
                            for c in range(2):
                                nc.vector.tensor_mul(
                                    oT[c], ops[:, GQ * c:GQ * (c + 1)],
                                    rbc_sb[c])
                        if pend is not None:
                            _emit_outproj(nc, psM, pout, wp, bp, out_d, pend,
                                          f32, AF)
                        pend = (oT, tail["g"], tail["bt"])
                    tail = {"probs": probs, "vb": vb, "g": g, "bt": bt}

                    # drip next block's projection chunks into this group's
                    # chain-idle PE slivers
                    if gen_next is not None:
                        for _ in range(6):
                            if next(gen_next, "end") == "end":
                                gen_next = None
                                break

                # drain any leftover projection chunks, hand over block state
                while gen_next is not None:
                    if next(gen_next, "end") == "end":
                        gen_next = None
                if bt + 1 < BT_PER_CORE:
                    blk = blk_next
            # drain: tail of the final group, then the last two outprojs
            tprobs, tvb = tail["probs"], tail["vb"]
            zps = psZ.tile([8, GQ], f32, tag="z", name="psz")
            for b in range(4):
                nc.tensor.matmul(zps, ez[:, 8 * b:8 * (b + 1)],
                                 tprobs[:, GQ * b:GQ * (b + 1)],
                                 start=(b == 0), stop=(b == 3))
            rz_f = pz.tile([8, GQ], f32, tag="zf", name="rzf")
            nc.vector.reciprocal_approx_fast(out=rz_f, in_=zps)
            z_sb = pz.tile([8, GQ], bf16, tag="z", name="zsb")
            with nc.allow_low_precision(reason="bf16 1/Z"):
                nc.scalar.copy(z_sb, rz_f)
            ops = psAV.tile([128, 2 * GQ], f32, tag="ops", name="psops")
            for c in range(2):
                for li in range(GRP):
                    for pb in (2 * c, 2 * c + 1):
                        nc.tensor.matmul(
                            ops[64 * (pb % 2):64 * (pb % 2 + 1),
                                GQ * c + 64 * li:GQ * c + 64 * (li + 1)],
                            tvb[:, 256 * li + 64 * pb:256 * li + 64 * (pb + 1)],
                            tprobs[:, GQ * pb + 64 * li:GQ * pb + 64 * (li + 1)],
                            start=True, stop=True)
            rbc = [psM.tile([128, GQ], f32, tag="mm", name="psrbc")
                   for _ in range(2)]
            for c in range(2):
                nc.tensor.matmul(rbc[c], ebc_bf[:, 128 * c:128 * (c + 1)],
                                 z_sb, start=True, stop=True)
            rbc_sb = [po.tile([128, GQ], f32, tag="rbcsb", name="rbcsb")
                      for _ in range(2)]
            nc.scalar.copy(rbc_sb[0], rbc[0])
            nc.vector.tensor_copy(rbc_sb[1], rbc[1])
            oT = [po.tile([128, GQ], bf16, tag="oT", name="oT")
                  for _ in range(2)]
            with nc.allow_low_precision(reason="bf16 o"):
                for c in range(2):
                    nc.vector.tensor_mul(oT[c], ops[:, GQ * c:GQ * (c + 1)],
                                         rbc_sb[c])
            _emit_outproj(nc, psM, pout, wp, bp, out_d, pend, f32, AF)
            _emit_outproj(nc, psM, pout, wp, bp, out_d,
                          (oT, tail["g"], tail["bt"]), f32, AF)
    nc.compile()
    return nc


def _emit_outproj(nc, psM, pout, wp, bp, out_d, pend, f32, AF):
    oT, g, bt = pend
    for mc in range(2):
        ps = psM.tile([128, GQ], f32, tag="mm", name="psproj")
        for kc in range(2):
            nc.tensor.matmul(ps, wp[kc][mc], oT[kc],
                             start=(kc == 0), stop=(kc == 1))
        osb = pout.tile([128, GQ], f32, tag="out", name="outsb")
        if mc == 0:
            nc.scalar.activation(osb, ps, AF.Identity,
                                 bias=bp[:, 0:1], scale=1.0)
        else:
            nc.vector.tensor_scalar_add(osb, ps, bp[:, 1:2])
        nc.sync.dma_start(
            out=out_d[bt, 128 * mc:128 * (mc + 1), GQ * g:GQ * (g + 1)],
            in_=osb)


def _host_inputs(x, relative_bias, w_qkv, b_qkv, w_proj, b_proj):
    scale = D ** -0.5
    wq = w_qkv[:, :C] * scale          # [256, 256]
    wk = w_qkv[:, C:2 * C]
    wv = w_qkv[:, 2 * C:]
    bqv = b_qkv[:C] * scale            # [256]
    bv = b_qkv[2 * C:]
    wqk_full = np.concatenate([wq, wk], axis=1)        # [256, 512]
    wqk = np.stack([wqk_full[:128], wqk_full[128:]]).astype(np.float32)
    wvs = np.stack([wv[:128], wv[128:]]).astype(np.float32)
    # oT feature order: chunk0 = [h0, h4, h1, h5], chunk1 = [h2, h6, h3, h7]
    perm = np.concatenate([np.arange(32 * h, 32 * (h + 1))
                           for h in (0, 4, 1, 5, 2, 6, 3, 7)])
    w_proj_p = w_proj[perm, :]
    wp = np.zeros((2, 2, 128, 128), np.float32)
    for kc in range(2):
        for mc in range(2):
            wp[kc, mc] = w_proj_p[128 * kc:128 * (kc + 1),
                                  128 * mc:128 * (mc + 1)]
    bq = np.stack([bqv[:128], bqv[128:]], axis=1).astype(np.float32)  # [128,2]
    bpv = bv @ w_proj + b_proj                                       # [256]
    bp = np.stack([bpv[:128], bpv[128:]], axis=1).astype(np.float32)
    # exp(bias) tiles: expb[p, 512*b + 64*li + q] = exp(bias[head(p,b), q, k=p%64])
    expb = np.zeros((128, 2048), np.float32)
    eb = np.exp(relative_bias.astype(np.float64)).astype(np.float32)  # [8, q, k]
    for b in range(4):
        t = np.zeros((128, 64), np.float32)
        t[0:64, :] = eb[b].T          # rows k, cols q
        t[64:128, :] = eb[b + 4].T
        expb[:, 512 * b:512 * (b + 1)] = np.tile(t, (1, 8))
    ez = np.zeros((128, 32), np.float32)
    for b in range(4):
        ez[0:64, 8 * b + 2 * b] = 1        # head b    -> z row 2b
        ez[64:128, 8 * b + 2 * b + 1] = 1  # head b+4 -> z row 2b+1
    ebc = np.zeros((8, 256), np.float32)
    for r in range(8):
        ebc[r, 32 * r:32 * (r + 1)] = 1.0
    import ml_dtypes
    st = np.zeros((128, STW), np.float32)
    st[:, 0:512] = wqk[0]
    st[:, 512:1024] = wqk[1]
    st[:, 1024:1280] = wvs[0]
    st[:, 1280:1536] = wvs[1]
    for i in range(2):
        for j in range(2):
            st[:, 1536 + 256 * i + 128 * j:1536 + 256 * i + 128 * (j + 1)] = wp[i, j]
    st[:, 2048:4096] = expb
    st[:, 4096:4128] = ez
    st[0:8, 4128:4384] = ebc
    stf = np.concatenate([bq, bp], axis=1).astype(np.float32)  # [128, 4]
    return dict(statics=st.astype(ml_dtypes.bfloat16), staticf=stf)


def kernel(x, relative_bias, w_qkv, b_qkv, w_proj, b_proj):
    import sys
    if '/opt/trn_rl_repo' not in sys.path:
        sys.path.insert(0, '/opt/trn_rl_repo')
    from concourse.bass_utils import run_bass_kernel_spmd

    x = np.asarray(x, np.float32)
    const = _host_inputs(np.asarray(x, np.float32),
                         np.asarray(relative_bias, np.float32),
                         np.asarray(w_qkv, np.float32),
                         np.asarray(b_qkv, np.float32),
                         np.asarray(w_proj, np.float32),
                         np.asarray(b_proj, np.float32))
    import ml_dtypes
    xr = np.ascontiguousarray(x.reshape(NBT, C, HW)).astype(ml_dtypes.bfloat16)
    nc = _build_bass()
    in_maps = []
    for c in range(NCORES):
        m = dict(const)
        m["x"] = np.ascontiguousarray(xr[c * BT_PER_CORE:(c + 1) * BT_PER_CORE])
        in_maps.append(m)
    res = run_bass_kernel_spmd(nc, in_maps, list(range(NCORES)))
    global LAST_RESULT
    LAST_RESULT = res
    outs = res.results
    out = np.concatenate([o["out"].reshape(BT_PER_CORE, C, HW) for o in outs],
                         axis=0)
    return out.reshape(B, T, C, H, W).astype(np.float32)
